# revision 1
# baseline (speedup 1.0000x reference)
"""Trainium2 Bass kernel for nn_MaskedSelfAttention (causal, QK rms-norm).

Sharding: 8 cores = 2 (batch) x 4 (head groups of 4 heads).
Each core computes qkv projection for its heads, causal flash-style
attention (no max subtraction -- scores are bounded by rms norm), and a
partial FC output over its heads' feature slice. Host sums the 4 partials
per batch.

Self-contained: hardcodes shapes from the problem spec.
"""

import numpy as np

import concourse.bacc as bacc
import concourse.mybir as mybir
import concourse.tile as tile
from concourse.bass_utils import run_bass_kernel_spmd

B, L, D = 2, 2048, 1024
DH = 64
NH = D // DH            # 16 heads total
P = 128
NHC = 4                 # heads per core
E3 = 3 * NHC * DH       # 768 qkv rows per core
LB = L // P             # 16 l-blocks
KB = D // P             # 8 contraction blocks
EPS = 1e-5
F32 = mybir.dt.float32
F32R = mybir.dt.float32r


def R(ap):
    return ap.bitcast(F32R)

FX = mybir.ActivationFunctionType
MULT = mybir.AluOpType.mult

_CACHE = {}


def _build_nc():
    nc = bacc.Bacc("TRN2", target_bir_lowering=False, debug=False)

    xT = nc.dram_tensor("xT", (D, L), F32, kind="ExternalInput").ap()
    wqkvT = nc.dram_tensor("wqkvT", (D, E3), F32, kind="ExternalInput").ap()
    wfcT = nc.dram_tensor("wfcT", (NHC * DH, D), F32, kind="ExternalInput").ap()
    triu = nc.dram_tensor("triu", (P, P), F32, kind="ExternalInput").ap()
    wqk = nc.dram_tensor("wqk", (P, 1), F32, kind="ExternalInput").ap()
    ident = nc.dram_tensor("ident", (P, P), F32, kind="ExternalInput").ap()
    sel = nc.dram_tensor("sel", (NHC, NHC * DH), F32, kind="ExternalInput").ap()
    outp = nc.dram_tensor("outp", (L, D), F32, kind="ExternalOutput").ap()

    with tile.TileContext(nc) as tc:
        with (
            tc.tile_pool(name="cpool", bufs=1) as cpool,
            tc.tile_pool(name="wpool", bufs=1) as wpool,
            tc.tile_pool(name="ppool", bufs=1) as ppool,
            tc.tile_pool(name="xpool", bufs=3) as xpool,
            tc.tile_pool(name="work", bufs=8) as work,
            tc.tile_pool(name="ptpool", bufs=6) as ptpool,
            tc.tile_pool(name="opool", bufs=3) as opool,
        ):
            ident_sb = cpool.tile([P, P], F32)
            nc.sync.dma_start(ident_sb, ident)
            identr_sb = cpool.tile([P, P], F32R)
            nc.sync.dma_start(identr_sb, R(ident))
            triu_sb = cpool.tile([P, P], F32)
            nc.sync.dma_start(triu_sb, triu)
            wqk_sb = cpool.tile([P, 1], F32)
            nc.sync.dma_start(wqk_sb, wqk)
            sel_sb = cpool.tile([NHC, NHC * DH], F32R)
            nc.sync.dma_start(sel_sb, R(sel))
            biasq = cpool.tile([P, 1], F32)
            nc.vector.memset(biasq, DH * EPS)

            wqkv_sb = wpool.tile([P, KB, E3], F32R)
            wqkvT_r = wqkvT.rearrange("(ko p) e -> p ko e", p=P)
            nc.sync.dma_start(wqkv_sb[:, 0:1, :], R(wqkvT_r[:, 0:1, :]))
            nc.sync.dma_start(wqkv_sb[:, 1:KB, :], R(wqkvT_r[:, 1:KB, :]))
            wfc_sb = wpool.tile([P, 2, D], F32R)
            nc.sync.dma_start(wfc_sb, R(wfcT.rearrange("(g p) e -> p g e", p=P)))

            # persistent activations (per-partition fp32 bytes in comments)
            qT = ppool.tile([P, 2, L], F32R)      # 16KB  [dh-pair, hp, l]
            kT = ppool.tile([P, 2, L], F32R)      # 16KB
            vext = ppool.tile([P, LB, NHC, DH + 1], F32R)  # 16.25KB, col DH = ones
            oT = ppool.tile([P, 2, L], F32R)      # 16KB  unnorm O^T, normed in place
            rec = ppool.tile([NHC, L], F32R)      # 1/denom, free-major
            dnT = ppool.tile([P, LB, NHC], F32)    # denom, lq-partition-major
            recT = ppool.tile([P, LB, NHC], F32)

            onesf = cpool.tile([P, 1], F32)
            nc.vector.memset(onesf, 1.0)
            nc.vector.tensor_copy(
                vext[:, :, :, DH : DH + 1],
                onesf[:, :, None, None].to_broadcast((P, LB, NHC, 1)),
            )

            # ---- Phase A: qkv projection (l,e') + rms norm + transpose q,k ----
            with (
                tc.tile_pool(name="psA", bufs=2, space="PSUM") as psA,
                tc.tile_pool(name="psT", bufs=3, space="PSUM") as psT,
            ):
                for m in range(LB):
                    # whole xT column-block for this m in one DMA: [128, 8, 128]
                    xc = xpool.tile([P, KB, P], F32R, tag="xc", name=f"xc_{m}")
                    nc.sync.dma_start(
                        xc, R(xT.rearrange("(ko p) l -> p ko l", p=P)[:, :, m * P : (m + 1) * P])
                    )
                    xts = [xc[:, k, :] for k in range(KB)]
                    # q+k in one 512-wide matmul chain, v in a 256-wide one
                    ps = psA.tile([P, 2 * NHC * DH], F32, tag="qkps", bufs=3, name=f"qkps_{m}")
                    psv = psA.tile([P, NHC * DH], F32, tag="vps", bufs=2, name=f"vps_{m}")
                    for k in range(KB):
                        nc.tensor.matmul(
                            ps,
                            lhsT=xts[k],
                            rhs=wqkv_sb[:, k, 0 : 2 * NHC * DH],
                            start=(k == 0),
                            stop=(k == KB - 1),
                        )
                    for k in range(KB):
                        nc.tensor.matmul(
                            psv,
                            lhsT=xts[k],
                            rhs=wqkv_sb[:, k, 2 * NHC * DH : 3 * NHC * DH],
                            start=(k == 0),
                            stop=(k == KB - 1),
                        )
                    nc.scalar.copy(
                        vext[:, m, :, 0:DH],
                        psv.rearrange("p (h d) -> p h d", d=DH),
                    )
                    sq = work.tile([P, 2 * NHC * DH], F32, tag="sq", name=f"sq_{m}")
                    nc.scalar.activation(sq, ps, FX.Square)
                    ssq = work.tile([P, 2 * NHC], F32, tag="ssq", name=f"ssq_{m}")
                    nc.vector.reduce_sum(
                        ssq,
                        sq.rearrange("p (h d) -> p h d", d=DH),
                        axis=mybir.AxisListType.X,
                    )
                    rin = work.tile([P, 2 * NHC], F32, tag="rin", name=f"rin_{m}")
                    # both q,k: 1/rin = 0.125 / sqrt(mean + eps); the extra 1/64
                    # vs the reference's 1/8 sdpa scale is undone by exp(scale=8)
                    nc.scalar.activation(rin, ssq, FX.Sqrt, bias=biasq[:, :], scale=1.0)
                    inv = work.tile([P, 2 * NHC], F32, tag="inv", name=f"inv_{m}")
                    nc.vector.reciprocal(inv, rin)
                    qn = work.tile([P, 2 * NHC * DH], F32R, tag="qn", name=f"qn_{m}")
                    nc.vector.tensor_tensor(
                        qn.rearrange("p (h d) -> p h d", d=DH),
                        ps.rearrange("p (h d) -> p h d", d=DH),
                        inv[:, :, None].to_broadcast((P, 2 * NHC, DH)),
                        MULT,
                    )
                    for g in range(4):  # blocks: 0,1 -> qT; 2,3 -> kT
                        dst = qT if g < 2 else kT
                        tp = psT.tile([P, P], F32R, tag="tp", name=f"tp_{m}_{g}")
                        nc.tensor.transpose(tp, qn[:, g * P : (g + 1) * P], identr_sb)
                        if g % 2 == 0:
                            nc.vector.tensor_copy(dst[:, g % 2, m * P : (m + 1) * P], tp)
                        else:
                            nc.scalar.copy(dst[:, g % 2, m * P : (m + 1) * P], tp)
                    # fold norm weights (q_norm_w * k_norm_w) into kT, per-partition
                    nc.vector.tensor_scalar_mul(
                        kT[:, :, m * P : (m + 1) * P], kT[:, :, m * P : (m + 1) * P], wqk_sb
                    )

            # ---- Phase B: attention. S^T = kT.T@qT, P^T = exp, O^T += V^T@P^T ----
            with (
                tc.tile_pool(name="psS", bufs=3, space="PSUM") as psS,
                tc.tile_pool(name="psO", bufs=2, space="PSUM") as psO,
            ):
                for hp in range(2):
                    for c in range(4):
                        oTps = [
                            psO.tile([DH + 1, 512], F32, tag="oT", name=f"oT_{hp}_{c}_{h2}")
                            for h2 in range(2)
                        ]
                        nj = 4 * c + 4
                        for j in range(nj):
                            off = max(0, j * P - c * 512)
                            W = 512 - off
                            st = psS.tile([P, 2, 512], F32, tag="sT", name=f"sT_{hp}_{c}_{j}")
                            for h2 in range(2):
                                nc.tensor.matmul(
                                    st[:, h2, 0:W],
                                    lhsT=kT[h2 * DH : (h2 + 1) * DH, hp, j * P : (j + 1) * P],
                                    rhs=qT[h2 * DH : (h2 + 1) * DH, hp, c * 512 + off : (c + 1) * 512],
                                    start=True,
                                    stop=True,
                                )
                            pt = ptpool.tile([P, 2, 512], F32R, tag="pt", name=f"pt_{hp}_{c}_{j}")
                            nc.scalar.activation(pt[:, :, 0:W], st[:, :, 0:W], FX.Exp, scale=8.0)
                            if j >= 4 * c:
                                nc.vector.tensor_tensor(
                                    pt[:, :, 0:P],
                                    pt[:, :, 0:P],
                                    triu_sb[:, None, :].to_broadcast((P, 2, P)),
                                    MULT,
                                )
                            for h2 in range(2):
                                nc.tensor.matmul(
                                    oTps[h2][:, off:512],
                                    lhsT=vext[:, j, 2 * hp + h2, :],
                                    rhs=pt[:, h2, 0:W],
                                    start=(j == 0),
                                    stop=(j == nj - 1),
                                    skip_group_check=True,
                                )
                        for h2 in range(2):
                            lh = 2 * hp + h2
                            # stage denom row at partition 64 (no partition shift),
                            # then PE-transpose 128-col pieces to lq-partition-major
                            dnc = work.tile([DH + 1, 512], F32, tag="dnc", name=f"dnc_{hp}_{c}_{h2}")
                            nc.vector.tensor_copy(dnc[DH : DH + 1, :], oTps[h2][DH : DH + 1, :])
                            dnps = psO.tile([P, NHC], F32, tag="oT", name=f"dnps_{hp}_{c}_{h2}")
                            for mi in range(4):
                                nc.tensor.transpose(
                                    dnps[:, mi : mi + 1],
                                    dnc[DH : DH + 1, mi * P : (mi + 1) * P],
                                    ident_sb[DH : DH + 1, DH : DH + 1],
                                )
                            nc.vector.tensor_copy(dnT[:, 4 * c : 4 * c + 4, lh], dnps)
                            if h2 == 0:
                                nc.vector.tensor_copy(
                                    oT[h2 * DH : (h2 + 1) * DH, hp, c * 512 : (c + 1) * 512],
                                    oTps[h2][0:DH, :],
                                )
                            else:
                                nc.scalar.copy(
                                    oT[h2 * DH : (h2 + 1) * DH, hp, c * 512 : (c + 1) * 512],
                                    oTps[h2][0:DH, :],
                                )

            # ---- Phase C: reciprocal of denominators + normalize O^T + FC ----
            with (
                tc.tile_pool(name="psC", bufs=1, space="PSUM") as psC,
                tc.tile_pool(name="psR", bufs=3, space="PSUM") as psR,
                tc.tile_pool(name="psF", bufs=4, space="PSUM") as psF,
            ):
                nc.vector.reciprocal(
                    recT.rearrange("p a b -> p (a b)"),
                    dnT.rearrange("p a b -> p (a b)"),
                )
                for c in range(4):
                    for mi in range(4):
                        m = 4 * c + mi
                        tp2 = psC.tile([NHC, P], F32, tag="recb", name=f"recb_{m}")
                        nc.tensor.transpose(tp2, recT[:, m, :], ident_sb)
                        nc.vector.tensor_copy(rec[:, m * P : (m + 1) * P], tp2)
                    for hp in range(2):
                        for h2 in range(2):
                            lh = 2 * hp + h2
                            rb = psR.tile([DH, 512], F32, tag="rb", name=f"rb_{lh}_{c}")
                            nc.tensor.matmul(
                                rb,
                                lhsT=sel_sb[:, lh * DH : (lh + 1) * DH],
                                rhs=rec[:, c * 512 : (c + 1) * 512],
                                start=True,
                                stop=True,
                            )
                            seg = oT[h2 * DH : (h2 + 1) * DH, hp, c * 512 : (c + 1) * 512]
                            nc.vector.tensor_tensor(seg, seg, rb, MULT)
                    for mi in range(4):
                        m = 4 * c + mi
                        for n in range(2):
                            fp = psF.tile([P, 512], F32, tag="fc", name=f"fc_{m}_{n}")
                            for g in range(2):
                                nc.tensor.matmul(
                                    fp,
                                    lhsT=oT[:, g, m * P : (m + 1) * P],
                                    rhs=wfc_sb[:, g, n * 512 : (n + 1) * 512],
                                    start=(g == 0),
                                    stop=(g == 1),
                                )
                            ot = opool.tile([P, 512], F32, tag="ot", name=f"ot_{m}_{n}")
                            nc.scalar.copy(ot, fp)
                            nc.sync.dma_start(outp[m * P : (m + 1) * P, n * 512 : (n + 1) * 512], ot)

    nc.compile()
    return nc


def _make_in_maps(x, w_qkv, w_fc, q_norm_w, k_norm_w):
    triu_f = np.triu(np.ones((P, P), dtype=np.float32))
    ident = np.eye(P, dtype=np.float32)
    sel = np.kron(np.eye(NHC), np.ones((1, DH))).astype(np.float32)
    wqk = np.tile((q_norm_w * k_norm_w).astype(np.float32), 2).reshape(P, 1)
    wqkvT = {}
    wfcTs = {}
    for hg in range(4):
        h0 = hg * NHC
        rows = np.concatenate(
            [
                w_qkv[h0 * DH : (h0 + NHC) * DH],
                w_qkv[D + h0 * DH : D + (h0 + NHC) * DH],
                w_qkv[2 * D + h0 * DH : 2 * D + (h0 + NHC) * DH],
            ],
            axis=0,
        )
        wqkvT[hg] = np.ascontiguousarray(rows.T.astype(np.float32))
        wfcTs[hg] = np.ascontiguousarray(w_fc.T[h0 * DH : (h0 + NHC) * DH].astype(np.float32))
    xTs = [np.ascontiguousarray(x[b].T.astype(np.float32)) for b in range(B)]
    in_maps = []
    for core in range(8):
        b, hg = core // 4, core % 4
        in_maps.append(
            {
                "xT": xTs[b],
                "wqkvT": wqkvT[hg],
                "wfcT": wfcTs[hg],
                "triu": triu_f,
                "wqk": wqk,
                "ident": ident,
                "sel": sel,
            }
        )
    return in_maps


def _is_causal(mask):
    idx = np.arange(mask.shape[0])
    return mask.shape == (L, L) and bool(np.all(mask == (idx[None, :] <= idx[:, None])))


def _reference_numpy(x, mask, w_qkv, w_fc, q_norm_w, k_norm_w, subset_attention_size):
    # slow but general fallback (only used if mask is not causal)
    b, l, d = x.shape
    qkv = x @ w_qkv.T
    q, k, v = np.split(qkv, 3, axis=-1)

    def heads(t):
        return t.reshape(b, l, NH, DH).transpose(0, 2, 1, 3)

    def rms(t, w):
        return t * (1.0 / np.sqrt(np.mean(t * t, -1, keepdims=True) + EPS)) * w

    q, k, v = heads(q), heads(k), heads(v)
    q, k = rms(q, q_norm_w), rms(k, k_norm_w)

    def sdpa(q, k, v, m):
        s = np.einsum("bhqd,bhkd->bhqk", q, k) / np.sqrt(DH)
        s = np.where(m[None, None], s, -1e30)
        s = s - s.max(-1, keepdims=True)
        p = np.exp(s)
        p /= p.sum(-1, keepdims=True)
        return np.einsum("bhqk,bhkd->bhqd", p, v)

    S = int(subset_attention_size) if subset_attention_size is not None else None
    if S is not None and S < l:
        o = np.concatenate(
            [
                sdpa(q[:, :, :S], k[:, :, :S], v[:, :, :S], mask[:S, :S]),
                sdpa(q[:, :, S:], k, v, mask[S:, :]),
            ],
            axis=2,
        )
    else:
        o = sdpa(q, k, v, mask)
    o = o.transpose(0, 2, 1, 3).reshape(b, l, d)
    return (o @ w_fc.T).astype(np.float32)


def kernel(**inputs):
    x = np.asarray(inputs["x"], dtype=np.float32)
    mask = np.asarray(inputs["mask"])
    w_qkv = np.asarray(inputs["w_qkv"], dtype=np.float32)
    w_fc = np.asarray(inputs["w_fc"], dtype=np.float32)
    q_norm_w = np.asarray(inputs["q_norm_w"], dtype=np.float32)
    k_norm_w = np.asarray(inputs["k_norm_w"], dtype=np.float32)

    if not _is_causal(mask):
        return _reference_numpy(
            x, mask, w_qkv, w_fc, q_norm_w, k_norm_w, inputs.get("subset_attention_size")
        )

    if "nc" not in _CACHE:
        _CACHE["nc"] = _build_nc()
    nc = _CACHE["nc"]

    in_maps = _make_in_maps(x, w_qkv, w_fc, q_norm_w, k_norm_w)
    res = run_bass_kernel_spmd(nc, in_maps, core_ids=list(range(8)))
    parts = [res.results[i]["outp"] for i in range(8)]
    out = np.empty((B, L, D), dtype=np.float32)
    for b in range(B):
        acc = np.zeros((L, D), dtype=np.float64)
        for hg in range(4):
            acc += parts[b * 4 + hg]
        out[b] = acc.astype(np.float32)
    return out



# revision 11
# speedup vs baseline: 1.0935x; 1.0935x over previous
"""Trainium2 Bass kernel for nn_MaskedSelfAttention (causal, QK rms-norm).

Sharding: 8 cores = 2 (batch) x 4 (head groups of 4 heads).
Each core computes qkv projection for its heads, causal flash-style
attention (no max subtraction -- scores are bounded by rms norm), and a
partial FC output over its heads' feature slice. Host sums the 4 partials
per batch.

v4: bf16 datapath; projection runs as a separate up-front phase (the PE
power-PWM only engages once the activation engine starts streaming exp,
so projection gets full PE speed); attention, FC and the output DMA are
interleaved per query block; input DMAs spread across engine queues for
a fast start; denominator path releases attention PSUM immediately and
computes reciprocals lazily in a partition-parallel layout.

Self-contained: hardcodes shapes from the problem spec.
"""

import numpy as np

import concourse.bacc as bacc
import concourse.mybir as mybir
import concourse.tile as tile
from concourse.bass_utils import run_bass_kernel_spmd

B, L, D = 2, 2048, 1024
DH = 64
NH = D // DH
P = 128
NHC = 4                 # heads per core
E3 = 3 * NHC * DH       # 768 qkv rows per core
LB = L // P             # 16 l-blocks
KB = D // P             # 8 contraction blocks
NC = 4                  # query 512-blocks
EPS = 1e-5
F32 = mybir.dt.float32
F32R = mybir.dt.float32r
BF16 = mybir.dt.bfloat16


def R(ap):
    return ap.bitcast(F32R)

FX = mybir.ActivationFunctionType
MULT = mybir.AluOpType.mult

_CACHE = {}


def _build_nc():
    nc = bacc.Bacc("TRN2", target_bir_lowering=False, debug=False)

    xT = nc.dram_tensor("xT", (D, L), BF16, kind="ExternalInput").ap()
    wqkvT = nc.dram_tensor("wqkvT", (D, E3), BF16, kind="ExternalInput").ap()
    wfcT = nc.dram_tensor("wfcT", (NHC * DH, D), BF16, kind="ExternalInput").ap()
    triu = nc.dram_tensor("triu", (P, P), BF16, kind="ExternalInput").ap()
    ztriu = nc.dram_tensor("ztriu", (P, 2 * P), BF16, kind="ExternalInput").ap()
    wqk = nc.dram_tensor("wqk", (P, 1), F32, kind="ExternalInput").ap()
    ident = nc.dram_tensor("ident", (P, P), F32, kind="ExternalInput").ap()
    identb = nc.dram_tensor("identb", (P, P), BF16, kind="ExternalInput").ap()
    sel2 = nc.dram_tensor("sel2", (2, P), F32, kind="ExternalInput").ap()
    outp = nc.dram_tensor("outp", (L, D), F32, kind="ExternalOutput").ap()

    with tile.TileContext(nc) as tc:
        with (
            tc.tile_pool(name="cpool", bufs=1) as cpool,
            tc.tile_pool(name="wpool", bufs=1) as wpool,
            tc.tile_pool(name="ppool", bufs=1) as ppool,
            tc.tile_pool(name="xpool", bufs=6) as xpool,
            tc.tile_pool(name="work", bufs=2) as work,
            tc.tile_pool(name="ptpool", bufs=8) as ptpool,
            tc.tile_pool(name="opool", bufs=3) as opool,
            # psum: 4+2+2 banks, declared in order so every matmul target
            # stays 2KB-bank aligned
            tc.tile_pool(name="psBig", bufs=2, space="PSUM") as psBig,
            tc.tile_pool(name="psO", bufs=1, space="PSUM") as psO,
            tc.tile_pool(name="psTmp", bufs=2, space="PSUM") as psTmp,
        ):
            # ---- input DMAs, spread across engine queues for a fast start ----
            xT_r = xT.rearrange("(ko p) l -> p ko l", p=P)
            xcs = []
            xc0 = xpool.tile([P, KB, P], BF16, tag="xc", name="xc_0")
            nc.sync.dma_start(xc0, xT_r[:, :, 0:P])
            xcs.append(xc0)

            wqkv_sb = wpool.tile([P, KB, E3], BF16)
            wqkvT_r = wqkvT.rearrange("(ko p) e -> p ko e", p=P)
            for ko in range(KB):
                nc.gpsimd.dma_start(wqkv_sb[:, ko : ko + 1, :], wqkvT_r[:, ko : ko + 1, :])

            ident_sb = cpool.tile([P, P], F32)
            nc.scalar.dma_start(ident_sb, ident)
            identb_sb = cpool.tile([P, P], BF16)
            nc.scalar.dma_start(identb_sb, identb)
            triu_sb = cpool.tile([P, P], BF16)
            nc.scalar.dma_start(triu_sb, triu)
            ztriu_sb = cpool.tile([P, 2 * P], BF16)
            nc.scalar.dma_start(ztriu_sb, ztriu)
            wqk_sb = cpool.tile([P, 1], F32)
            nc.scalar.dma_start(wqk_sb, wqk)
            sel2_sb = cpool.tile([2, P], F32R)
            nc.scalar.dma_start(sel2_sb, R(sel2))
            biasq = cpool.tile([P, 1], F32)
            nc.vector.memset(biasq, DH * EPS)

            wfc_sb = wpool.tile([P, 2, D], BF16)
            nc.gpsimd.dma_start(wfc_sb, wfcT.rearrange("(g p) e -> p g e", p=P))

            for m in range(1, LB):
                xc = xpool.tile([P, KB, P], BF16, tag="xc", name=f"xc_{m}")
                nc.sync.dma_start(xc, xT_r[:, :, m * P : (m + 1) * P])
                xcs.append(xc)

            # persistent activations
            qT = ppool.tile([P, 2, L], BF16)      # [dh-pair, hp, l]
            kT = ppool.tile([P, 2, L], BF16)
            vext = ppool.tile([P, LB, NHC, DH + 1], BF16)  # col DH = ones

            onesf = cpool.tile([P, 1], BF16)
            nc.vector.memset(onesf, 1.0)
            nc.vector.tensor_copy(
                vext[:, :, :, DH : DH + 1],
                onesf[:, :, None, None].to_broadcast((P, LB, NHC, 1)),
            )

            # ---- Phase A: qkv projection (runs before any exp, unthrottled) ----
            for m in range(LB):
                xc = xcs[m]
                xts = [xc[:, k, :] for k in range(KB)]
                big = psBig.tile([P, 8 * P], F32, tag="big", name=f"pj_{m}")
                pqk = big[:, 0 : 2 * NHC * DH]
                pv = big[:, 2 * NHC * DH : 3 * NHC * DH]
                for k in range(KB):
                    nc.tensor.matmul(
                        pqk,
                        lhsT=xts[k],
                        rhs=wqkv_sb[:, k, 0 : 2 * NHC * DH],
                        start=(k == 0),
                        stop=(k == KB - 1),
                    )
                for k in range(KB):
                    nc.tensor.matmul(
                        pv,
                        lhsT=xts[k],
                        rhs=wqkv_sb[:, k, 2 * NHC * DH : 3 * NHC * DH],
                        start=(k == 0),
                        stop=(k == KB - 1),
                    )
                nc.scalar.copy(
                    vext[:, m, :, 0:DH],
                    pv.rearrange("p (h d) -> p h d", d=DH),
                )
                sq = work.tile([P, 2 * NHC * DH], F32, tag="sq", name=f"sq_{m}")
                nc.scalar.activation(sq, pqk, FX.Square)
                ssq = work.tile([P, 2 * NHC], F32, tag="ssq", name=f"ssq_{m}")
                nc.vector.reduce_sum(
                    ssq,
                    sq.rearrange("p (h d) -> p h d", d=DH),
                    axis=mybir.AxisListType.X,
                )
                rin = work.tile([P, 2 * NHC], F32, tag="rin", name=f"rin_{m}")
                # both q,k: 1/rin = 0.125 / sqrt(mean + eps); the extra 1/64
                # vs the reference's 1/8 sdpa scale is undone by exp(scale=8)
                nc.scalar.activation(rin, ssq, FX.Sqrt, bias=biasq[:, :], scale=1.0)
                inv = work.tile([P, 2 * NHC], F32, tag="inv", name=f"inv_{m}")
                nc.vector.reciprocal(inv, rin)
                qn = work.tile([P, 2 * NHC * DH], BF16, tag="qn", name=f"qn_{m}")
                nc.vector.tensor_tensor(
                    qn.rearrange("p (h d) -> p h d", d=DH),
                    pqk.rearrange("p (h d) -> p h d", d=DH),
                    inv[:, :, None].to_broadcast((P, 2 * NHC, DH)),
                    MULT,
                )
                tpq = psTmp.tile([P, 4 * P], BF16, tag="tmp", name=f"tp_{m}")
                for g in range(4):  # blocks: 0,1 -> qT; 2,3 -> kT
                    nc.tensor.transpose(
                        tpq[:, g * P : (g + 1) * P], qn[:, g * P : (g + 1) * P], identb_sb
                    )
                nc.vector.tensor_copy(
                    qT[:, :, m * P : (m + 1) * P],
                    tpq[:, 0 : 2 * P].rearrange("p (g l) -> p g l", g=2),
                )
                nc.scalar.copy(
                    kT[:, :, m * P : (m + 1) * P],
                    tpq[:, 2 * P : 4 * P].rearrange("p (g l) -> p g l", g=2),
                )
                # fold norm weights (q_norm_w * k_norm_w) into kT, per-partition
                nc.vector.tensor_scalar_mul(
                    kT[:, :, m * P : (m + 1) * P], kT[:, :, m * P : (m + 1) * P], wqk_sb
                )

            # ---- Phase B: attention + fc + output DMA, per query block ----
            for c in range(NC):
                oTc = opool.tile([P, 2, 512], BF16, tag="oTc", bufs=2, name=f"oTc_{c}")
                nj = 4 * c + 4
                for hp in range(2):
                    oTps = [
                        psO.tile([DH + 1, 512], F32, tag=f"o{h2}", bufs=1,
                                 name=f"oT_{hp}_{c}_{h2}")
                        for h2 in range(2)
                    ]
                    for j in range(nj):
                        if j < 4 * c:
                            off, W, mask = 0, 512, None
                        elif j == 4 * c:
                            off, W, mask = 0, 512, triu_sb
                        elif j == 4 * c + 1:
                            off, W, mask = 128, 384, triu_sb
                        elif j == 4 * c + 2:
                            off, W, mask = 256, 256, triu_sb
                        else:  # j == 4c+3: extend W to 256, mask = [zeros|triu]
                            off, W, mask = 256, 256, ztriu_sb
                        big = psBig.tile([P, 8 * P], F32, tag="big", name=f"st_{hp}_{c}_{j}")
                        st = big.rearrange("p (h w) -> p h w", w=512)
                        for h2 in range(2):
                            nc.tensor.matmul(
                                st[:, h2, 0:W],
                                lhsT=kT[h2 * DH : (h2 + 1) * DH, hp, j * P : (j + 1) * P],
                                rhs=qT[h2 * DH : (h2 + 1) * DH, hp, c * 512 + off : (c + 1) * 512],
                                start=True,
                                stop=True,
                            )
                        pt = ptpool.tile([P, 2, 512], BF16, tag="pt", name=f"pt_{hp}_{c}_{j}")
                        nc.scalar.activation(pt[:, :, 0:W], st[:, :, 0:W], FX.Exp, scale=8.0)
                        if mask is not None:
                            MW = mask.shape[1]
                            nc.vector.tensor_tensor(
                                pt[:, :, 0:MW],
                                pt[:, :, 0:MW],
                                mask[:, None, :].to_broadcast((P, 2, MW)),
                                MULT,
                            )
                        for h2 in range(2):
                            nc.tensor.matmul(
                                oTps[h2][:, off:512],
                                lhsT=vext[:, j, 2 * hp + h2, :],
                                rhs=pt[:, h2, 0:W],
                                start=(j == 0),
                                stop=(j == nj - 1),
                                skip_group_check=True,
                            )
                    # release oTps fast: copy denom rows + O rows out, then
                    # compute 1/denom lazily in a partition-parallel layout
                    dstage = work.tile([DH + 1, 2, 512], F32, tag="dstage",
                                       name=f"dstage_{hp}_{c}")
                    nc.vector.tensor_copy(dstage[DH : DH + 1, 0, :], oTps[0][DH : DH + 1, :])
                    nc.scalar.copy(dstage[DH : DH + 1, 1, :], oTps[1][DH : DH + 1, :])
                    nc.vector.tensor_copy(oTc[0:DH, hp, :], oTps[0][0:DH, :])
                    nc.scalar.copy(oTc[DH : 2 * DH, hp, :], oTps[1][0:DH, :])
                    # dn rows -> lq-partition-major, reciprocal, back to rows
                    dnps = psTmp.tile([P, 8], F32, tag="tmp", name=f"dnps_{hp}_{c}")
                    for h2 in range(2):
                        for mi in range(4):
                            nc.tensor.transpose(
                                dnps[:, 2 * mi + h2 : 2 * mi + h2 + 1],
                                dstage[DH : DH + 1, h2, mi * P : (mi + 1) * P],
                                ident_sb[DH : DH + 1, DH : DH + 1],
                            )
                    dinv = work.tile([P, 8], F32, tag="dinv", name=f"dinv_{hp}_{c}")
                    nc.vector.reciprocal(dinv, dnps)
                    rbr_ps = psTmp.tile([2, 512], F32, tag="tmp", name=f"rbr_{hp}_{c}")
                    for mi in range(4):
                        nc.tensor.transpose(
                            rbr_ps[:, mi * P : (mi + 1) * P],
                            dinv[:, 2 * mi : 2 * mi + 2],
                            ident_sb,
                        )
                    rbr = work.tile([2, 512], F32R, tag="rbr", name=f"rbr_{hp}_{c}")
                    if hp == 0:
                        nc.vector.tensor_copy(rbr, rbr_ps)
                    else:
                        nc.scalar.copy(rbr, rbr_ps)
                    for h2 in range(2):
                        rb = psTmp.tile([DH, 512], F32, tag="tmp",
                                        name=f"rb_{hp}_{c}_{h2}")
                        nc.tensor.matmul(
                            rb,
                            lhsT=sel2_sb[:, h2 * DH : (h2 + 1) * DH],
                            rhs=rbr,
                            start=True,
                            stop=True,
                        )
                        dst = oTc[h2 * DH : (h2 + 1) * DH, hp, :]
                        nc.vector.tensor_tensor(dst, dst, rb, MULT)

                # fc + output DMA for this query block
                for mi in range(4):
                    m = 4 * c + mi
                    big = psBig.tile([P, 8 * P], F32, tag="big", name=f"fc_{m}")
                    for n in range(2):
                        for g in range(2):
                            nc.tensor.matmul(
                                big[:, n * 512 : (n + 1) * 512],
                                lhsT=oTc[:, g, mi * P : (mi + 1) * P],
                                rhs=wfc_sb[:, g, n * 512 : (n + 1) * 512],
                                start=(g == 0),
                                stop=(g == 1),
                            )
                    ot = opool.tile([P, 8 * P], F32, tag="ot", name=f"ot_{m}")
                    nc.vector.tensor_copy(ot, big)
                    nc.gpsimd.dma_start(outp[m * P : (m + 1) * P, :], ot)

    nc.compile()
    return nc


def _make_in_maps(x, w_qkv, w_fc, q_norm_w, k_norm_w):
    import ml_dtypes
    bf = ml_dtypes.bfloat16
    triu_f = np.triu(np.ones((P, P), dtype=bf))
    ztriu_f = np.concatenate(
        [np.zeros((P, P), dtype=bf), np.triu(np.ones((P, P), dtype=bf))], axis=1
    )
    ident = np.eye(P, dtype=np.float32)
    identb = np.eye(P, dtype=bf)
    sel2 = np.kron(np.eye(2), np.ones((1, DH))).astype(np.float32)
    wqk = np.tile((q_norm_w * k_norm_w).astype(np.float32), 2).reshape(P, 1)
    wqkvT = {}
    wfcTs = {}
    for hg in range(4):
        h0 = hg * NHC
        rows = np.concatenate(
            [
                w_qkv[h0 * DH : (h0 + NHC) * DH],
                w_qkv[D + h0 * DH : D + (h0 + NHC) * DH],
                w_qkv[2 * D + h0 * DH : 2 * D + (h0 + NHC) * DH],
            ],
            axis=0,
        )
        wqkvT[hg] = np.ascontiguousarray(rows.T.astype(bf))
        wfcTs[hg] = np.ascontiguousarray(w_fc.T[h0 * DH : (h0 + NHC) * DH].astype(bf))
    xTs = [np.ascontiguousarray(x[b].T.astype(bf)) for b in range(B)]
    in_maps = []
    for core in range(8):
        b, hg = core // 4, core % 4
        in_maps.append(
            {
                "xT": xTs[b],
                "wqkvT": wqkvT[hg],
                "wfcT": wfcTs[hg],
                "triu": triu_f,
                "ztriu": ztriu_f,
                "wqk": wqk,
                "ident": ident,
                "identb": identb,
                "sel2": sel2,
            }
        )
    return in_maps


def _is_causal(mask):
    idx = np.arange(mask.shape[0])
    return mask.shape == (L, L) and bool(np.all(mask == (idx[None, :] <= idx[:, None])))


def _reference_numpy(x, mask, w_qkv, w_fc, q_norm_w, k_norm_w, subset_attention_size):
    # slow but general fallback (only used if mask is not causal)
    b, l, d = x.shape
    qkv = x @ w_qkv.T
    q, k, v = np.split(qkv, 3, axis=-1)

    def heads(t):
        return t.reshape(b, l, NH, DH).transpose(0, 2, 1, 3)

    def rms(t, w):
        return t * (1.0 / np.sqrt(np.mean(t * t, -1, keepdims=True) + EPS)) * w

    q, k, v = heads(q), heads(k), heads(v)
    q, k = rms(q, q_norm_w), rms(k, k_norm_w)

    def sdpa(q, k, v, m):
        s = np.einsum("bhqd,bhkd->bhqk", q, k) / np.sqrt(DH)
        s = np.where(m[None, None], s, -1e30)
        s = s - s.max(-1, keepdims=True)
        p = np.exp(s)
        p /= p.sum(-1, keepdims=True)
        return np.einsum("bhqk,bhkd->bhqd", p, v)

    S = int(subset_attention_size) if subset_attention_size is not None else None
    if S is not None and S < l:
        o = np.concatenate(
            [
                sdpa(q[:, :, :S], k[:, :, :S], v[:, :, :S], mask[:S, :S]),
                sdpa(q[:, :, S:], k, v, mask[S:, :]),
            ],
            axis=2,
        )
    else:
        o = sdpa(q, k, v, mask)
    o = o.transpose(0, 2, 1, 3).reshape(b, l, d)
    return (o @ w_fc.T).astype(np.float32)


def kernel(**inputs):
    x = np.asarray(inputs["x"], dtype=np.float32)
    mask = np.asarray(inputs["mask"])
    w_qkv = np.asarray(inputs["w_qkv"], dtype=np.float32)
    w_fc = np.asarray(inputs["w_fc"], dtype=np.float32)
    q_norm_w = np.asarray(inputs["q_norm_w"], dtype=np.float32)
    k_norm_w = np.asarray(inputs["k_norm_w"], dtype=np.float32)

    if not _is_causal(mask):
        return _reference_numpy(
            x, mask, w_qkv, w_fc, q_norm_w, k_norm_w, inputs.get("subset_attention_size")
        )

    if "nc" not in _CACHE:
        _CACHE["nc"] = _build_nc()
    nc = _CACHE["nc"]

    in_maps = _make_in_maps(x, w_qkv, w_fc, q_norm_w, k_norm_w)
    res = run_bass_kernel_spmd(nc, in_maps, core_ids=list(range(8)))
    parts = [res.results[i]["outp"] for i in range(8)]
    out = np.empty((B, L, D), dtype=np.float32)
    for b in range(B):
        acc = np.zeros((L, D), dtype=np.float64)
        for hg in range(4):
            acc += parts[b * 4 + hg]
        out[b] = acc.astype(np.float32)
    return out
